# revision 69
# baseline (speedup 1.0000x reference)
"""Trainium2 Bass kernel for AlexNet-trunk + top-2 MoE (nn_Net_89343909691624).

v2 strategy (8 NeuronCores):
  - Data-parallel conv trunk (8 images/core). conv1 folds kw-phase taps into
    the contraction dim (K=72 + 3 leftover taps) fed by ONE padded-image DMA
    per 14-row chunk (host pre-pads x), cutting both PE cycles and the DMA
    instruction count ~6x vs per-shift row loads.
  - Gate FC runs LOCALLY on each core's own 8 images (f32r, exact top-2
    semantics); logits are AllGathered (tiny, f32).
  - Features are gathered in bf16 (expert MLP tolerance), in 2 chunks
    overlapped with the trunk tail.
  - Expert-parallel MoE in bf16 with swapped-operand FC1 (features as the
    stationary operand, expert weights moving, N=288 per matmul).
  - Gate-weighted outputs ReduceScatter-add (bf16): core r emits rows
    8r:8r+8, so the SPMD gather IS the full [64,1000] output.
"""
import os

import numpy as np
import ml_dtypes

os.environ.setdefault("JAX_COMPILATION_CACHE_DIR", "/tmp/jaxcache")
try:
    import jax as _jax
    _jax.config.update("jax_compilation_cache_dir", "/tmp/jaxcache")
    _jax.config.update("jax_persistent_cache_min_entry_size_bytes", 0)
    _jax.config.update("jax_persistent_cache_min_compile_time_secs", 0.0)
except Exception:
    pass

from concourse import bacc, tile, mybir
from concourse import bass_utils  # noqa: F401  (canonical SPMD entry)

F32 = mybir.dt.float32
F32R = mybir.dt.float32r
BF16 = mybir.dt.bfloat16
AF = mybir.ActivationFunctionType
ALU = mybir.AluOpType
AX = mybir.AxisListType

NCORES = 8
IPC = 8      # images per core
PAIRS = IPC // 2
NQ = 2       # kw phase taps folded into K (K = 36*NQ); leftover taps = 11-4*NQ


# ---------------------------------------------------------------- host prep

def _prep_static(c1w, c2w, c3w, c4w, c5w, gw1,
                 c1b, c2b, c3b, c4b, c5b, gb1):
    """Core-independent weight rearrangement (shared by all cores)."""
    o = {}
    # conv1: rows kh=4*KH+dy (KH<3, dy<4), cols kw=4*q+r.
    c1p = np.zeros((64, 3, 12, 12), np.float32)
    c1p[:, :, :11, :11] = c1w
    t = c1p.reshape(64, 3, 3, 4, 3, 4).transpose(4, 3, 2, 1, 5, 0)
    # t: [q, dy, KH, c, r, o]; taps (dy=3, KH=2) hit the zero-pad kernel row
    # kh=11 (indices 33..35 of each q-block) and are dropped entirely
    o["w1f"] = np.ascontiguousarray(
        t[:NQ].reshape(NQ, 36, 4, 64)[:, 0:33].reshape(33 * NQ, 4, 64))
    if 4 * NQ < 11:
        o["w1x"] = np.ascontiguousarray(
            t[NQ, :, :, :, 0:3, :].reshape(36, 3, 64)[0:33])
    else:
        o["w1x"] = np.zeros((33, 3, 64), np.float32)
    # conv2: kh-pair folding. partitions (d,c): p = d*64+c
    t = c2w.transpose(1, 2, 3, 0)  # [c64, kh5, kw5, o192]
    o["w2a"] = np.ascontiguousarray(
        np.concatenate([t[:, 0:4:2], t[:, 1:5:2]], axis=0))  # [128, khp2, kw5, 192]
    o["w2b"] = np.ascontiguousarray(t[:, 4])                 # [64, kw5, 192]
    t = c3w.transpose(1, 2, 3, 0).reshape(192, 9, 384)
    o["w3a"] = np.ascontiguousarray(t[:128])
    o["w3b"] = np.ascontiguousarray(t[128:])
    t = c4w.transpose(1, 2, 3, 0).reshape(384, 9, 256)
    o["w4t"] = np.ascontiguousarray(t.reshape(3, 128, 9, 256).transpose(1, 0, 2, 3))
    t = c5w.transpose(1, 2, 3, 0).reshape(256, 9, 256)
    o["w5t"] = np.ascontiguousarray(t.reshape(2, 128, 9, 256).transpose(1, 0, 2, 3))
    # gate FC1 weights (streamed at trunk tail, f32r)
    o["gw1r"] = np.ascontiguousarray(
        gw1.reshape(2, 128, 36, 72).transpose(1, 0, 2, 3))   # [128, kc2, s36, 72]
    # packed constants [128, 86] f32: biases + esel(core-dep, later) + ident
    cst = np.zeros((128, 86), np.float32)
    cst[0:64, 0] = c1b
    cst[0:128, 1] = c2b[:128]
    cst[0:64, 2] = c2b[128:]
    cst[:, 3:6] = c3b.reshape(3, 128).T
    cst[:, 6:8] = c4b.reshape(2, 128).T
    cst[:, 8:10] = c5b.reshape(2, 128).T
    cst[0:72, 10] = gb1
    cst[0:64, 22:86] = np.eye(64, dtype=np.float32)
    o["_cst"] = cst
    return o


def _prep_expert(gw2, gb2, ew1, eb1, ew2, eb2, ew3, eb3, r, cst_base):
    """Expert-r weights + per-core constants."""
    o = {}
    o["ew1b"] = np.ascontiguousarray(
        ew1[r].reshape(2, 128, 36, 288).transpose(1, 0, 2, 3)
    ).astype(ml_dtypes.bfloat16)                              # [128, 2, 36, 288]
    o["eb1b"] = eb1[r].reshape(1, 288).astype(ml_dtypes.bfloat16)
    # FC2 weights arranged for transposed output: e2t[p, m, oc, o] =
    # ew2[m*96+p, oc*72+o]
    e2 = ew2[r]
    e2t = np.ascontiguousarray(
        e2.reshape(3, 96, 2, 72).transpose(1, 0, 2, 3))
    o["e2t"] = e2t.astype(ml_dtypes.bfloat16)
    e3 = ew3[r]
    e3p = np.zeros((73, 2008), np.float32)
    e3p[0:72, 0:1000] = e3[0:72]
    e3p[72, 0:1000] = eb3[r]
    e3p[0:72, 1000:2000] = e3[72:144]
    e3p[0:72, 2000:2008] = gw2
    e3p[72, 2000:2008] = gb2
    o["e3p"] = e3p
    cst = cst_base.copy()
    cst[0:72, 11] = eb2[r][0:72]
    cst[0:72, 12] = eb2[r][72:144]
    cst[0:64, 14:22] = 0.0
    cst[0:64, 14 + r] = 1.0
    o["consts"] = cst
    return o


def _prep_xk(x):
    """conv1 input, tap-replicated on host: xk[i, (q,dy,KH,c), y', v] =
    xpad[i, c, 4*y'+4*KH+dy, v+4*q] — so each 14-row chunk is ONE 3-dim DMA."""
    n = x.shape[0]
    xp = np.zeros((n, 3, 232, 240), np.float32)
    xp[:, :, 2:226, 2:226] = x
    s = xp.strides
    v = np.lib.stride_tricks.as_strided(
        xp,
        shape=(n, NQ, 4, 3, 3, 55, 232),
        strides=(s[0], 4 * s[3], s[2], 4 * s[2], s[1], 4 * s[2], s[3]))
    # drop the dead kh=11 taps (last 3 of each 36-index q-block)
    v = v.reshape(n, NQ, 36, 55, 232)[:, :, 0:33]
    return np.ascontiguousarray(v.reshape(n, 33 * NQ, 55, 232))


# ---------------------------------------------------------------- AP helpers

def win3s2(ap, Ho, Wo, rowstep):
    """Overlapping 3x3/stride-2 window view [C, Ho, Wo, 3, 3] of a [C, H, W] AP."""
    w = ap.copy()
    part = w.ap[0]
    w.ap = mybir.VecI64Pair(
        [list(part), [2 * rowstep, Ho], [2, Wo], [rowstep, 3], [1, 3]])
    return w


def _ap_custom(base_ap, dims, extra_offset=0):
    """Clone base_ap with explicit [stride, count] dims and offset shift."""
    c = base_ap.copy()
    c.ap = mybir.VecI64Pair([list(d) for d in dims])
    c.offset = c.offset + extra_offset
    return c





# ---------------------------------------------------------------- builder

def build_nc(single=False):
    nc = bacc.Bacc("TRN2", target_bir_lowering=False, debug=False,
                   num_devices=1 if single else NCORES)

    def inp(name, shape, dt=F32R):
        return nc.dram_tensor(name, list(shape), dt, kind="ExternalInput").ap()

    xk = inp("xk", [IPC, 33 * NQ, 55, 232])
    w1f = inp("w1f", [33 * NQ, 4, 64])
    w1x = inp("w1x", [33, 3, 64])
    w2a = inp("w2a", [128, 2, 5, 192])
    w2b = inp("w2b", [64, 5, 192])
    w3a = inp("w3a", [128, 9, 384])
    w3b = inp("w3b", [64, 9, 384])
    w4t = inp("w4t", [128, 3, 9, 256])
    w5t = inp("w5t", [128, 2, 9, 256])
    gw1r = inp("gw1r", [128, 2, 36, 72])
    ew1b = inp("ew1b", [128, 2, 36, 288], BF16)
    eb1b = inp("eb1b", [1, 288], BF16)
    e2t = inp("e2t", [96, 3, 2, 72], BF16)
    e3p = inp("e3p", [73, 2008])
    consts = inp("consts", [128, 86], F32)

    out = nc.dram_tensor("out", [IPC, 1000], BF16, kind="ExternalOutput").ap()

    with tile.TileContext(nc) as tc:
        _build_body(nc, tc, locals(), single=single)
    nc.compile()
    return nc


def _build_body(nc, tc, io, single=False):
    shared = "Local" if single else "Shared"
    with (
        tc.tile_pool(name="wts", bufs=1) as wts,
        tc.tile_pool(name="per", bufs=1) as per,
        tc.tile_pool(name="gwp", bufs=3) as gwp,
        tc.tile_pool(name="dram", bufs=1, space="DRAM") as dram,
        tc.tile_pool(name="h1p", bufs=1, space="PSUM") as h1p,
    ):
        # ---- resident weights / constants
        CONSTS = wts.tile([128, 86], F32, tag="CONSTS")
        W1 = wts.tile([33 * NQ, 4, 64], F32R, tag="W1")
        W1X = wts.tile([33, 3, 64], F32R, tag="W1X")
        W2A = wts.tile([128, 2, 5, 192], F32R, tag="W2A")
        W2B = wts.tile([64, 5, 192], F32R, tag="W2B")
        W3A = wts.tile([128, 9, 384], F32R, tag="W3A")
        W3B = wts.tile([64, 9, 384], F32R, tag="W3B")
        E2T = wts.tile([96, 3, 2, 72], BF16, tag="E2T")
        GW2B = wts.tile([73, 8], F32R, tag="GW2B")
        EW1 = wts.tile([128, 2, 36, 288], BF16, tag="EW1")
        EB1 = wts.tile([1, 288], BF16, tag="EB1")
        ONE1 = wts.tile([1, 64], BF16, tag="ONE1")

        C1B = CONSTS[0:64, 0:1]
        GB1 = CONSTS[0:72, 10:11]
        ESEL = CONSTS[0:64, 14:22]
        IDENT = CONSTS[0:64, 22:86]

        # local features (f32r for exact gate; bf16 copy for the MoE gather)
        FL = per.tile([128, 2, IPC, 6, 6], F32R, tag="FL")
        FLb = per.tile([128, 2, IPC, 6, 6], BF16, tag="FLb")
        G1L = per.tile([73, IPC], F32R, tag="G1L")
        LL = per.tile([IPC, 8], F32, tag="LL")
        # first-half gathered features (loaded mid-trunk, feeds FC1 chain A)
        FAbA = [per.tile([128, 32, 36], BF16, tag=f"FAbA{kc}", name=f"FAbA{kc}")
                for kc in range(2)]

        # startup loads (x chunk DMAs for image 0 go first on their queues;
        # conv2+ weights follow behind pair-0 conv1 emission)
        nc.scalar.dma_start(W1[:], io["w1f"])
        nc.scalar.dma_start(W1X[:], io["w1x"])
        nc.sync.dma_start(CONSTS[:], io["consts"])
        nc.vector.memset(ONE1[:], 1.0)

        def _late_weight_loads():
            nc.scalar.dma_start(W2A[:], io["w2a"])
            nc.scalar.dma_start(W2B[:], io["w2b"])
            nc.scalar.dma_start(W3A[:], io["w3a"])
            nc.scalar.dma_start(W3B[:], io["w3b"])
            nc.sync.dma_start(E2T[:], io["e2t"])
            nc.sync.dma_start(GW2B[:], io["e3p"][0:73, 2000:2008])
            nc.sync.dma_start(EB1[:], io["eb1b"])

        # DRAM bounce buffers for collectives
        feat_loc = dram.tile([IPC, 9216], BF16)
        feat_gA = dram.tile([32, 9216], BF16, addr_space=shared)
        feat_gB = dram.tile([32, 9216], BF16, addr_space=shared)
        logit_loc = dram.tile([IPC, 8], F32)
        logit_all = dram.tile([64, 8], F32, addr_space=shared)
        ar_in = [dram.tile([64, 500], BF16, name=f"ar_in{n}") for n in range(2)]
        ar_out = [dram.tile([IPC, 500], BF16, name=f"ar_out{n}") for n in range(2)]

        GT = []

        def _fab_load(FAt, gsrc, kc, eng=None):
            src = _ap_custom(gsrc[:, :], [[36, 128], [9216, 32], [1, 36]],
                             extra_offset=kc * 128 * 36)
            (eng or nc.sync).dma_start(FAt[:], src)

        with (
            tc.tile_pool(name="acts", bufs=1) as acts,
            tc.tile_pool(name="x36p", bufs=2) as x36p,
            tc.tile_pool(name="w45p", bufs=2) as w45p,
        ):
            # persistent padded activation tiles (borders zeroed once)
            P1D2 = [acts.tile([128, 2, 31, 32], F32R, tag="P1D", name="P1D")
                    for k in range(1)] * 2
            P2 = [(acts.tile([128, 2, 15, 16], F32R, tag=f"P2a{k}", name=f"P2a{k}"),
                   acts.tile([64, 2, 15, 16], F32R, tag=f"P2b{k}", name=f"P2b{k}"))
                  for k in range(2)]
            P3 = [acts.tile([128, 2, 15, 16], F32R, tag=f"P3_{m}", name=f"P3_{m}")
                  for m in range(3)]
            P4 = [acts.tile([128, 2, 15, 16], F32R, tag=f"P4_{m}", name=f"P4_{m}")
                  for m in range(2)]
            for t in [P1D2[0]] + [p for pq in P2 for p in pq] + P3 + P4:
                nc.vector.memset(t[:].bitcast(F32), 0.0)

            with (
                tc.tile_pool(name="c345ps", bufs=3, space="PSUM") as c345ps,
            ):
                with (
                    tc.tile_pool(name="c1ps", bufs=2, space="PSUM") as c1ps,
                    tc.tile_pool(name="c2ps", bufs=1, space="PSUM") as c2ps,
                ):
                    # software pipeline: conv1/conv2 of pair p+1 are
                    # emitted inside the conv3-5 window of pair p, spreading
                    # the conv1 input DMA burst under PE-heavy phases.
                    _conv1_pair(nc, io, 0, x36p, acts, W1, W1X, CONSTS,
                                c1ps, P1D2, _late_weight_loads)
                    _conv2_pair(nc, io, 0, acts, W2A, W2B, CONSTS, c2ps,
                                P1D2, P2)
                    for p in range(PAIRS):
                        _conv345_pair(nc, io, p, acts, w45p, P2, P3, P4,
                                      W3A, W3B, CONSTS, c345ps,
                                      FL, FLb, feat_loc,
                                      nxt=None if p == PAIRS - 1 else dict(
                                          x36p=x36p, W1=W1, W1X=W1X,
                                          c1ps=c1ps, P1D2=P1D2, c2ps=c2ps,
                                          W2A=W2A, W2B=W2B))
                        if p == 1:
                            _gather_feats(nc, feat_loc, feat_gA, 0, single)
                            # prefetch expert weights (half now, half at p==2)
                            nc.scalar.dma_start(EW1[:, 0], io["ew1b"][:, 0])
                        if p == 2:
                            nc.scalar.dma_start(EW1[:, 1], io["ew1b"][:, 1])
                            # chain-A features + gate weights (consumed at tail)
                            for kc in range(2):
                                _fab_load(FAbA[kc], feat_gA, kc)
                            for kc in range(2):
                                for sb in range(3):
                                    gt = gwp.tile([128, 12, 72], F32R,
                                                  tag="gw1", name="gt")
                                    GT.append(gt)
                                    nc.gpsimd.dma_start(
                                        gt[:],
                                        io["gw1r"][:, kc, sb * 12:(sb + 1) * 12, :])
                    _gather_feats(nc, feat_loc, feat_gB, 1, single)

                # ---- trunk tail: local gate (f32r, exact) + FC1 chain A
                h1ps = h1p.tile([64, 288], F32, tag="h1ps")
                with tc.tile_pool(name="gps_pool", bufs=1, space="PSUM") as gpp:
                    gps = gpp.tile([72, IPC], F32, tag="gps")
                    # interleave SEQ-bound gate matmuls (N=8) with engine-bound
                    # FC1 chain-A matmuls (N=288): sequencer and PE overlap
                    first = True
                    for kc in range(2):
                        for sb in range(3):
                            gt = GT[kc * 3 + sb]
                            for si in range(12):
                                s = sb * 12 + si
                                sx, sy = divmod(s, 6)
                                nc.tensor.matmul(
                                    gps[:], gt[:, si, :], FL[:, kc, :, sx, sy],
                                    start=first, stop=(kc == 1 and s == 35))
                                first = False
                                if s % 2 == 0:
                                    nc.tensor.matmul(
                                        h1ps[0:32, :],
                                        FAbA[kc][:, :, s],
                                        EW1[:, kc, s, :],
                                        start=(kc == 0 and s == 0), stop=False)
                    nc.vector.memset(G1L[:].bitcast(F32), 1.0)
                    nc.scalar.activation(G1L[0:72, :], gps[:], AF.Relu, bias=GB1)
                    lps = gpp.tile([IPC, 8], F32, tag="lps")
                    nc.tensor.matmul(lps[:], G1L[:], GW2B[:],
                                     start=True, stop=True)
                    nc.scalar.activation(LL[:], lps[:], AF.Copy)
                    nc.sync.dma_start(logit_loc[:], LL[:])
                    if single:
                        for r in range(NCORES):
                            nc.sync.dma_start(logit_all[IPC * r:IPC * (r + 1), :],
                                              logit_loc[:])
                    else:
                        nc.gpsimd.collective_compute(
                            "AllGather", ALU.bypass,
                            replica_groups=[list(range(NCORES))],
                            ins=[logit_loc[:].opt()],
                            outs=[logit_all[:].opt()],
                        )
                    # FC1 chain A remainder (odd s slices)
                    for kc in range(2):
                        for s in range(1, 36, 2):
                            nc.tensor.matmul(
                                h1ps[0:32, :], FAbA[kc][:, :, s], EW1[:, kc, s, :],
                                start=False, stop=False)

        # ---------------- MoE phase ----------------
        with (
            tc.tile_pool(name="moe", bufs=1) as moe,
            tc.tile_pool(name="mps", bufs=1, space="PSUM") as mps,
        ):
            # FC3 weights land here once trunk SBUF frees up
            E3P = moe.tile([73, 2008], F32R, tag="E3P")
            nc.scalar.dma_start(E3P[:], io["e3p"])
            # second-half gathered features -> FC1 chain B
            FAbB = [moe.tile([128, 32, 36], BF16, tag=f"FAbB{kc}", name=f"FAbB{kc}")
                    for kc in range(2)]
            _fab_load(FAbB[0], feat_gB, 0)
            _fab_load(FAbB[1], feat_gB, 1, eng=nc.scalar)
            for kc in range(2):
                for s in range(36):
                    nc.tensor.matmul(
                        h1ps[32:64, :], FAbB[kc][:, :, s], EW1[:, kc, s, :],
                        start=(kc == 0 and s == 0), stop=False)
            nc.tensor.matmul(h1ps[:], ONE1[:], EB1[:], start=False, stop=True)
            H1 = moe.tile([64, 288], F32, tag="H1")

            # relu in 3 chunks so each transpose starts as soon as its third
            # of H1 is ready
            H1T = [moe.tile([96, 64], BF16,
                            tag=f"H1T{m}", name=f"H1T{m}") for m in range(3)]
            for m in range(3):
                nc.scalar.activation(H1[:, 96 * m:96 * (m + 1)],
                                     h1ps[:, 96 * m:96 * (m + 1)], AF.Relu)
                tps = mps.tile([96, 64], F32, tag="tps")
                nc.tensor.transpose(tps[:], H1[:, 96 * m:96 * (m + 1)], IDENT)
                nc.scalar.activation(H1T[m][:], tps[:], AF.Copy)

            # ---- FC2 with transposed output (no H2 transpose round-trip):
            # h2T[oc] = relu(sum_m ew2-slice.T @ h1T[m] + eb2-slice)
            H2T = [moe.tile([73 if c == 0 else 72, 64], F32R,
                            tag=f"H2T{c}", name=f"H2T{c}") for c in range(2)]
            nc.vector.memset(H2T[0][:].bitcast(F32), 1.0)
            for oc in range(2):
                h2tps = mps.tile([72, 64], F32, tag=f"h2tps{oc}",
                                 name=f"h2tps{oc}")
                for m in range(3):
                    nc.tensor.matmul(h2tps[:], E2T[:, m, oc, :], H1T[m][:],
                                     start=(m == 0), stop=(m == 2))
                nc.scalar.activation(H2T[oc][0:72, :], h2tps[:], AF.Relu,
                                     bias=CONSTS[0:72, 11 + oc:12 + oc])

            # ---- top-2 softmax gate column from gathered logits (bb order)
            L = moe.tile([64, 8], F32, tag="L")
            for half in range(2):
                src = _ap_custom(logit_all[:, :], [[64, 8], [8, 4], [1, 8]],
                                 extra_offset=32 * half)
                nc.sync.dma_start(L[32 * half:32 * half + 32, :], src)
            v1 = moe.tile([64, 1], F32, tag="v1")
            nc.vector.tensor_reduce(out=v1[:], in_=L[:], axis=AX.X, op=ALU.max)
            nv1 = moe.tile([64, 1], F32, tag="nv1")
            nc.vector.tensor_scalar_mul(nv1[:], v1[:], -1.0)
            m1 = moe.tile([64, 8], F32, tag="m1")
            nc.vector.tensor_scalar(out=m1[:], in0=L[:], scalar1=v1[:], scalar2=None,
                                    op0=ALU.is_equal)
            lm = moe.tile([64, 8], F32, tag="lm")
            nc.vector.scalar_tensor_tensor(out=lm[:], in0=m1[:], scalar=-1e30,
                                           in1=L[:], op0=ALU.mult, op1=ALU.add)
            v2 = moe.tile([64, 1], F32, tag="v2")
            nc.vector.tensor_reduce(out=v2[:], in_=lm[:], axis=AX.X, op=ALU.max)
            e2v = moe.tile([64, 1], F32, tag="e2v")
            nc.scalar.activation(e2v[:], v2[:], AF.Exp, bias=nv1[:])
            den = moe.tile([64, 1], F32, tag="den")
            nc.vector.tensor_scalar_add(den[:], e2v[:], 1.0)
            inv = moe.tile([64, 1], F32, tag="inv")
            nc.vector.reciprocal(inv[:], den[:])
            expl = moe.tile([64, 8], F32, tag="expl")
            nc.scalar.activation(expl[:], L[:], AF.Exp, bias=nv1[:])
            msk = moe.tile([64, 8], F32, tag="msk")
            nc.vector.tensor_scalar(out=msk[:], in0=L[:], scalar1=v2[:], scalar2=None,
                                    op0=ALU.is_ge)
            t1 = moe.tile([64, 8], F32, tag="t1")
            nc.vector.tensor_tensor(out=t1[:], in0=msk[:], in1=expl[:], op=ALU.mult)
            t2 = moe.tile([64, 8], F32, tag="t2")
            nc.vector.tensor_tensor(out=t2[:], in0=t1[:], in1=ESEL, op=ALU.mult)
            gs = moe.tile([64, 1], F32, tag="gs")
            nc.vector.tensor_reduce(out=gs[:], in_=t2[:], axis=AX.X, op=ALU.add)
            gate = moe.tile([64, 1], F32, tag="gate")
            nc.vector.tensor_scalar(out=gate[:], in0=gs[:], scalar1=inv[:],
                                    scalar2=None, op0=ALU.mult)

            # ---- FC3 + gate scale -> ar_in (bf16)
            for nchunk in range(2):
                nlo = nchunk * 500
                eps = mps.tile([64, 500], F32, tag="eps", bufs=2, name="eps")
                nc.tensor.matmul(eps[:], H2T[0][:], E3P[0:73, nlo:nlo + 500],
                                 start=True, stop=False)
                nc.tensor.matmul(eps[:], H2T[1][:],
                                 E3P[0:72, 1000 + nlo:1000 + nlo + 500],
                                 start=False, stop=True)
                eps_s = moe.tile([64, 500], BF16, tag="eps_s", bufs=2, name="eps_s")
                nc.scalar.activation(eps_s[:], eps[:], AF.Copy, scale=gate[:])
                # un-permute bb -> b while storing (row b = 8r + 4*half + i)
                for half in range(2):
                    dst = _ap_custom(ar_in[nchunk][:, :],
                                     [[4000, 8], [500, 4], [1, 500]],
                                     extra_offset=4 * half * 500)
                    nc.sync.dma_start(dst, eps_s[32 * half:32 * half + 32, :])
                # ReduceScatter this half while the other half computes
                if single:
                    nc.sync.dma_start(ar_out[nchunk][:], ar_in[nchunk][0:IPC, :])
                else:
                    nc.gpsimd.collective_compute(
                        "ReduceScatter", ALU.add,
                        replica_groups=[list(range(NCORES))],
                        ins=[ar_in[nchunk][:].opt()],
                        outs=[ar_out[nchunk][:].opt()],
                    )
                nc.sync.dma_start(
                    _ap_custom(io["out"], [[1000, IPC], [1, 500]],
                               extra_offset=nlo),
                    ar_out[nchunk][:])


def _gather_feats(nc, feat_loc, dst, half, single):
    src = feat_loc[4 * half:4 * half + 4, :]
    if single:
        for r in range(NCORES):
            nc.sync.dma_start(dst[4 * r:4 * (r + 1), :], src)
    else:
        nc.gpsimd.collective_compute(
            "AllGather", ALU.bypass,
            replica_groups=[list(range(NCORES))],
            ins=[src.opt()],
            outs=[dst[:].opt()],
        )


def _conv1_pair(nc, io, p, x36p, acts,
                W1, W1X, CONSTS, c1ps, P1D2, after_first_conv1=None):
    k = p % 2
    P1D = P1D2[k]
    C1B = CONSTS[0:64, 0:1]

    # ---- conv1 (K=72 folded taps + 3 leftover kw taps) + pool1
    engs = (nc.sync, nc.scalar, nc.gpsimd)
    for j in range(2):
        i = 2 * p + j
        A1 = acts.tile([64, 55, 55], F32R, tag="A1", name="A1", bufs=2)
        for t in range(4):
            nk = 14 if t < 3 else 13
            xt = x36p.tile([33 * NQ, 14, 232], F32R, tag="XT", name="XT")
            engs[(4 * i + t) % 3].dma_start(
                xt[:, 0:nk, :], io["xk"][i, :, 14 * t:14 * t + nk, :])
            for (s0, sr) in [(0, 7), (7, nk - 7)]:
                nxk = max(0, 11 - 4 * NQ)
                ps = c1ps.tile([64, 7, 56], F32, tag="c1")
                for r in range(4):
                    nc.tensor.matmul(
                        ps[:, 0:sr, :], W1[:, r, :],
                        xt[:, s0:s0 + sr, r:r + 221:4],
                        start=(r == 0), stop=(nxk == 0 and r == 3))
                for e in range(nxk):
                    nc.tensor.matmul(
                        ps[:, 0:sr, :], W1X[:, e, :],
                        xt[0:33, s0:s0 + sr, 8 + e:8 + e + 221:4],
                        start=False, stop=(e == nxk - 1))
                nc.scalar.activation(
                    A1[:, 14 * t + s0:14 * t + s0 + sr, :], ps[:, 0:sr, 0:55],
                    AF.Relu, bias=C1B)
        if after_first_conv1 is not None and j == 0:
            after_first_conv1()
            after_first_conv1 = None
        nc.vector.tensor_reduce(
            out=P1D[0:64, j, 2:29, 2:29],
            in_=win3s2(A1[:].bitcast(F32), 27, 27, 55),
            axis=AX.XY, op=ALU.max)
        # per-image row-shifted replica for conv2's kh-pair fold, so
        # conv2(j=0) does not transitively wait on pool1(j=1)
        nc.sync.dma_start(
            P1D[64:128, j, 0:30, :].rearrange("c h w -> c (h w)"),
            P1D[0:64, j, :, :].rearrange("c h w -> c (h w)")[:, 32:992])



def _conv2_pair(nc, io, p, acts, W2A, W2B, CONSTS, c2ps, P1D2, P2):
    k = p % 2
    P1D = P1D2[k]
    P2a, P2b = P2[k]
    # ---- conv2 + pool2
    for j in range(2):
        A2a = acts.tile([128, 27, 27], F32R, tag="A2a", name="A2a", bufs=2)
        A2b = acts.tile([64, 27, 27], F32R, tag="A2b", name="A2b", bufs=2)
        for rc, (y0, rows) in enumerate([(0, 14), (14, 13)]):
            for mc, (msz, A2) in enumerate([(128, A2a), (64, A2b)]):
                mlo = mc * 128
                ps = c2ps.tile([msz, 14, 28], F32, tag=f"c2{mc}")
                first = True
                for khp in range(2):
                    for kw in range(5):
                        nc.tensor.matmul(
                            ps[:, 0:rows, :],
                            W2A[:, khp, kw, mlo:mlo + msz],
                            P1D[:, j, y0 + 2 * khp:y0 + 2 * khp + rows, kw:kw + 28],
                            start=first, stop=False)
                        first = False
                for kw in range(5):
                    nc.tensor.matmul(
                        ps[:, 0:rows, :],
                        W2B[:, kw, mlo:mlo + msz],
                        P1D[0:64, j, y0 + 4:y0 + 4 + rows, kw:kw + 28],
                        start=False, stop=(kw == 4))
                nc.scalar.activation(A2[:, y0:y0 + rows, :], ps[:, 0:rows, 0:27],
                                     AF.Relu, bias=CONSTS[0:msz, 1 + mc:2 + mc])
        nc.vector.tensor_reduce(
            out=P2a[:, j, 1:14, 1:14],
            in_=win3s2(A2a[:].bitcast(F32), 13, 13, 27),
            axis=AX.XY, op=ALU.max)
        nc.vector.tensor_reduce(
            out=P2b[:, j, 1:14, 1:14],
            in_=win3s2(A2b[:].bitcast(F32), 13, 13, 27),
            axis=AX.XY, op=ALU.max)



def _conv345_pair(nc, io, p, acts, w45p, P2, P3, P4,
                  W3A, W3B, CONSTS, c345ps, FL, FLb, feat_loc, nxt=None):
    P2a, P2b = P2[p % 2]
    # ---- conv3
    for mc in range(3):
        ps = c345ps.tile([128, 2, 13, 14], F32, tag="c345")
        first = True
        for kh in range(3):
            for kw in range(3):
                nc.tensor.matmul(
                    ps[:], W3A[:, kh * 3 + kw, mc * 128:(mc + 1) * 128],
                    P2a[:, :, kh:kh + 13, kw:kw + 14],
                    start=first, stop=False)
                first = False
                nc.tensor.matmul(
                    ps[:], W3B[:, kh * 3 + kw, mc * 128:(mc + 1) * 128],
                    P2b[:, :, kh:kh + 13, kw:kw + 14],
                    start=False, stop=(kh == 2 and kw == 2))
        nc.scalar.activation(P3[mc][:, :, 1:14, 1:14], ps[:, :, :, 0:13],
                             AF.Relu, bias=CONSTS[:, 3 + mc:4 + mc])

    if nxt is not None:
        _conv1_pair(nc, io, p + 1, nxt["x36p"], acts, nxt["W1"], nxt["W1X"],
                    CONSTS, nxt["c1ps"], nxt["P1D2"])

    # ---- conv4 (weights streamed per kc)
    ps4 = [c345ps.tile([128, 2, 13, 14], F32, tag="c345", name=f"c4ps{m}")
           for m in range(2)]
    first = True
    for kc in range(3):
        w4s = w45p.tile([128, 9, 256], F32R, tag="w45s", name="w4s")
        nc.scalar.dma_start(w4s[:], io["w4t"][:, kc, :, :])
        for kh in range(3):
            for kw in range(3):
                for mc in range(2):
                    nc.tensor.matmul(
                        ps4[mc][:], w4s[:, kh * 3 + kw, mc * 128:(mc + 1) * 128],
                        P3[kc][:, :, kh:kh + 13, kw:kw + 14],
                        start=first, stop=(kh == 2 and kw == 2 and kc == 2))
                first = False
    for mc in range(2):
        nc.scalar.activation(P4[mc][:, :, 1:14, 1:14], ps4[mc][:, :, :, 0:13],
                             AF.Relu, bias=CONSTS[:, 6 + mc:7 + mc])

    if nxt is not None:
        _conv2_pair(nc, io, p + 1, acts, nxt["W2A"], nxt["W2B"], CONSTS,
                    nxt["c2ps"], nxt["P1D2"], P2)

    # ---- conv5 + pool3 -> FL (f32r) and FLb (bf16) + feat store
    ps5 = [c345ps.tile([128, 2, 13, 14], F32, tag="c345", name=f"c5ps{m}")
           for m in range(2)]
    first = True
    for kc in range(2):
        w5s = w45p.tile([128, 9, 256], F32R, tag="w45s", name="w5s")
        nc.scalar.dma_start(w5s[:], io["w5t"][:, kc, :, :])
        for kh in range(3):
            for kw in range(3):
                for mc in range(2):
                    nc.tensor.matmul(
                        ps5[mc][:], w5s[:, kh * 3 + kw, mc * 128:(mc + 1) * 128],
                        P4[kc][:, :, kh:kh + 13, kw:kw + 14],
                        start=first, stop=(kh == 2 and kw == 2 and kc == 1))
                first = False
    for mc in range(2):
        A5 = acts.tile([128, 2, 13, 13], F32R, tag=f"A5_{mc}", name=f"A5_{mc}")
        nc.scalar.activation(A5[:], ps5[mc][:, :, :, 0:13], AF.Relu,
                             bias=CONSTS[:, 8 + mc:9 + mc])
        for j in range(2):
            nc.vector.tensor_reduce(
                out=FL[:, mc, 2 * p + j, :, :],
                in_=win3s2(A5[:, j].bitcast(F32), 6, 6, 13),
                axis=AX.XY, op=ALU.max)
    # bf16 copy of this pair's features
    nc.scalar.activation(
        FLb[:, :, 2 * p:2 * p + 2, :, :].rearrange("c k b x y -> c k b (x y)"),
        FL[:, :, 2 * p:2 * p + 2, :, :].bitcast(F32)
        .rearrange("c k b x y -> c k b (x y)"),
        AF.Copy)
    # store to feat_loc[b, (kc,c,s)] (partition-major pairing, one DMA per kc)
    for kc in range(2):
        dst = _ap_custom(feat_loc[:, :],
                         [[36, 128], [9216, 2], [1, 36]],
                         extra_offset=2 * p * 9216 + kc * 128 * 36)
        nc.sync.dma_start(
            dst,
            FLb[:, kc, 2 * p:2 * p + 2, :, :].rearrange("c b x y -> c b (x y)"))


# ---------------------------------------------------------------- runner

_CACHE = {}


def _get_state():
    if "nc" not in _CACHE:
        _CACHE["nc"] = build_nc()
    return _CACHE["nc"]


def _get_runner():
    """Cached jitted SPMD executor (device mesh over 8 cores, no donation)."""
    if "runner" in _CACHE:
        return _CACHE["runner"]
    import jax
    from jax.sharding import Mesh, PartitionSpec
    from jax.experimental.shard_map import shard_map
    from concourse.bass2jax import (_bass_exec_p, install_neuronx_cc_hook,
                                    partition_id_tensor)

    nc = _get_state()
    install_neuronx_cc_hook()
    partition_name = nc.partition_id_tensor.name if nc.partition_id_tensor else None
    in_names, out_names, out_avals, zero_outs = [], [], [], []
    for alloc in nc.m.functions[0].allocations:
        if not isinstance(alloc, mybir.MemoryLocationSet):
            continue
        name = alloc.memorylocations[0].name
        if alloc.kind == "ExternalInput":
            if name != partition_name:
                in_names.append(name)
        elif alloc.kind == "ExternalOutput":
            shape = tuple(alloc.tensor_shape)
            dtype = mybir.dt.np(alloc.dtype)
            out_names.append(name)
            out_avals.append(jax.core.ShapedArray(shape, dtype))
            zero_outs.append(np.zeros(shape, dtype))
    all_in = in_names + out_names + ([partition_name] if partition_name else [])

    def _body(*args):
        operands = list(args)
        if partition_name is not None:
            operands.append(partition_id_tensor())
        return tuple(_bass_exec_p.bind(
            *operands, out_avals=tuple(out_avals), in_names=tuple(all_in),
            out_names=tuple(out_names), lowering_input_output_aliases=(),
            sim_require_finite=True, sim_require_nnan=True, nc=nc))

    devices = jax.devices()[:NCORES]
    mesh = Mesh(np.asarray(devices), ("core",))
    nin = len(in_names) + len(out_names)
    fn = jax.jit(shard_map(_body, mesh=mesh,
                           in_specs=(PartitionSpec("core"),) * nin,
                           out_specs=(PartitionSpec("core"),) * len(out_names),
                           check_rep=False), keep_unused=True)
    _CACHE["runner"] = (fn, in_names, out_names, zero_outs)
    return _CACHE["runner"]


def _make_in_maps(inputs):
    static = _prep_static(
        inputs["c1w"], inputs["c2w"], inputs["c3w"], inputs["c4w"],
        inputs["c5w"], inputs["gw1"],
        inputs["c1b"], inputs["c2b"], inputs["c3b"], inputs["c4b"],
        inputs["c5b"], inputs["gb1"])
    cst_base = static.pop("_cst")
    xk = _prep_xk(np.asarray(inputs["x"], np.float32))
    in_maps = []
    for r in range(NCORES):
        m = dict(static)
        m.update(_prep_expert(inputs["gw2"], inputs["gb2"],
                              inputs["ew1"], inputs["eb1"], inputs["ew2"],
                              inputs["eb2"], inputs["ew3"], inputs["eb3"],
                              r, cst_base))
        m["xk"] = np.ascontiguousarray(xk[IPC * r:IPC * (r + 1)])
        in_maps.append({k: np.asarray(v) for k, v in m.items()})
    return in_maps


def kernel(**inputs):
    inputs = {k: np.asarray(v) for k, v in inputs.items()}
    fn, in_names, out_names, zero_outs = _get_runner()
    in_maps = _make_in_maps(inputs)
    concat_in = [np.concatenate([np.asarray(in_maps[c][n])
                                 for c in range(NCORES)], axis=0)
                 for n in in_names]
    concat_zero = [np.zeros((NCORES * z.shape[0], *z.shape[1:]), z.dtype)
                   for z in zero_outs]
    outs = fn(*concat_in, *concat_zero)
    oi = out_names.index("out")
    return np.asarray(outs[oi]).astype(np.float32)
